# revision 1
# baseline (speedup 1.0000x reference)
"""Causal self-attention (B=4, S=2048, D=2048, H=16) on 8 Trainium2 cores.

Sharding: core c -> (batch b = c//2, head-half = c%2, i.e. 8 of 16 heads).
Megatron-style: Wq/Wk/Wv column-parallel (8 heads' rows), Wo row-parallel
(matching 1024 columns).  Each core emits a partial (S, D) output for its
batch; host sums the two half partials per batch and adds bo.

All matmuls run as float32r (full-rate fp32 streaming on the PE, tf32-like
operand rounding, fp32 PSUM accumulation).  End-to-end L2 rel err ~3e-4.

Device pipeline per core (S=2048, DK=128, 8 local heads):
  Stage 1 (two 4-head passes): QKV projections.
    Q^T,K^T per head in [dk, s] layout; V in [s, dv] layout -> DRAM spill
    (per-head tensors so stage 2 can start as soon as a head is ready).
  Stage 2 per q-chunk (512), heads software-pipelined by two:
    S^T tile [k,q] = K^T_tile.T @ Q^T  (single dk=128 contraction)
    expS = Exp(S^T / sqrt(dk))  (no max-subtraction; scores ~ N(0,1))
    causal masking: in-place gpsimd affine_select on diagonal tiles
    denominator: DVE accumulate over k-chunks + all-ones matmul partition
    reduce (broadcasts over partitions for free) + reciprocal
    ctx^T [dv, q] = sum_k V_tile.T @ expS, normalized on PSUM->SBUF move
    out-proj: out[s,e] partial = sum_m ctx^T_slice.T @ WoT
"""

import math

import numpy as np

import concourse.bass as bass
import concourse.mybir as mybir
from concourse.bass_utils import run_bass_kernel_spmd
from concourse.tile import TileContext

B, S, D, H = 4, 2048, 2048, 16
DK = 128
NCORES = 8
HPC = H // 2          # 8 heads per core
MLOC = HPC * DK       # 1024 local head dims

F32 = mybir.dt.float32
F32R = mybir.dt.float32r
AF = mybir.ActivationFunctionType


def split_excess_waits(nc, max_waits=1):
    """walrus in this container accepts at most one sem-wait per instruction;
    move excess waits onto wait-only EventSemaphore insts inserted before."""
    ctr = 0
    for f in nc.m.functions:
        for bb in f.blocks:
            new = []
            changed = False
            for inst in bb.instructions:
                si = inst.sync_info
                if si is not None and si.on_wait and len(si.on_wait) > max_waits:
                    changed = True
                    waits = list(si.on_wait)
                    for w in waits[:-max_waits]:
                        ctr += 1
                        ev = mybir.InstEventSemaphore(
                            name=f"waitsplit-{ctr}", ins=[], outs=[],
                            sync_info=mybir.SyncInfo(on_wait=[w], on_update=[]))
                        ev.engine = inst.engine
                        new.append(ev)
                    si.on_wait = waits[-max_waits:]
                new.append(inst)
            if changed:
                bb.instructions = new
    return ctr


def build_nc(seq=S, gp_mask=True):
    """One core's program: full attention for 1 batch x 8 heads."""
    assert seq % 512 == 0
    NSC = seq // 512          # 512-wide s/q chunks
    NKC = seq // 128          # 128-wide k chunks
    SCALE = 1.0 / math.sqrt(DK)

    nc = bass.Bass()
    xt = nc.declare_dram_parameter("xt", [D, seq], F32R, isOutput=False)
    wqt = nc.declare_dram_parameter("wqt", [D, MLOC], F32R, isOutput=False)
    wkt = nc.declare_dram_parameter("wkt", [D, MLOC], F32R, isOutput=False)
    wvt = nc.declare_dram_parameter("wvt", [D, MLOC], F32R, isOutput=False)
    wot = nc.declare_dram_parameter("wot", [MLOC, D], F32R, isOutput=False)
    bqt = nc.declare_dram_parameter("bqt", [DK, HPC], F32, isOutput=False)
    bkt = nc.declare_dram_parameter("bkt", [DK, HPC], F32, isOutput=False)
    bvv = nc.declare_dram_parameter("bvv", [MLOC], F32, isOutput=False)
    ones = nc.declare_dram_parameter("ones", [DK, DK], F32R, isOutput=False)
    out = nc.declare_dram_parameter("out", [seq, D], F32, isOutput=True)

    xt_r = xt.rearrange("(dc p) s -> p dc s", p=128)      # [128, 16, seq]
    wqt_r = wqt.rearrange("(dc p) m -> p dc m", p=128)    # [128, 16, 1024]
    wkt_r = wkt.rearrange("(dc p) m -> p dc m", p=128)
    wvt_r = wvt.rearrange("(dc p) m -> p dc m", p=128)
    wot_r = wot.rearrange("(hc p) e -> p hc e", p=128)    # [128, 8, 2048]

    with TileContext(nc) as tc:
        with tc.tile_pool(name="dram", bufs=1, space="DRAM") as dpool, \
             tc.tile_pool(name="const", bufs=1) as cpool:
            # per-head spill tensors -> fine-grained stage1->stage2 deps
            qd = [dpool.tile([128, seq], F32R, name=f"qd{h}") for h in range(HPC)]
            kd = [dpool.tile([128, seq], F32R, name=f"kd{h}") for h in range(HPC)]
            vd = [dpool.tile([128, NKC, 128], F32R, name=f"vd{h}") for h in range(HPC)]

            ones_sb = cpool.tile([DK, DK], F32R)
            nc.sync.dma_start(out=ones_sb[:], in_=ones[:])
            ones_f = cpool.tile([DK, 2], F32)
            nc.vector.memset(ones_f[:], 1.0)
            if not gp_mask:
                masks = cpool.tile([128, 4, 512], F32)
                nc.gpsimd.memset(masks[:], 1.0)
                for j in range(4):
                    nc.gpsimd.affine_select(
                        out=masks[:, j, :], in_=masks[:, j, :],
                        compare_op=mybir.AluOpType.is_ge, fill=0.0,
                        base=-128 * j, pattern=[[1, 512]], channel_multiplier=-1)

            # ---------------- Stage 1: QKV projections ----------------
            with tc.tile_pool(name="s1w", bufs=1) as wpool, \
                 tc.tile_pool(name="s1x", bufs=2) as xpool, \
                 tc.tile_pool(name="s1s", bufs=4) as spool, \
                 tc.tile_pool(name="s1qk", bufs=4, space="PSUM") as qkp, \
                 tc.tile_pool(name="s1v", bufs=2, space="PSUM") as vps:
                bq_sb = cpool.tile([DK, HPC], F32)
                nc.sync.dma_start(out=bq_sb[:], in_=bqt[:])
                bk_sb = cpool.tile([DK, HPC], F32)
                nc.sync.dma_start(out=bk_sb[:], in_=bkt[:])
                bv_sb = cpool.tile([128, 2, 512], F32)
                for p_ in range(2):
                    nc.sync.dma_start(
                        out=bv_sb[:, p_, :],
                        in_=bvv[p_*512:(p_+1)*512].partition_broadcast(128))
                for p_ in range(2):           # head-half pass: heads 4p..4p+3
                    wq_sb = wpool.tile([128, 16, 512], F32R, tag="wq")
                    nc.sync.dma_start(out=wq_sb[:], in_=wqt_r[:, :, p_*512:(p_+1)*512])
                    wk_sb = wpool.tile([128, 16, 512], F32R, tag="wk")
                    nc.sync.dma_start(out=wk_sb[:], in_=wkt_r[:, :, p_*512:(p_+1)*512])
                    wv_sb = wpool.tile([128, 16, 512], F32R, tag="wv")
                    nc.sync.dma_start(out=wv_sb[:], in_=wvt_r[:, :, p_*512:(p_+1)*512])
                    for sc in range(NSC):
                        xin = xpool.tile([128, 16, 512], F32R, tag="xin")
                        nc.sync.dma_start(out=xin[:], in_=xt_r[:, :, sc*512:(sc+1)*512])
                        for hh in range(4):
                            h = p_ * 4 + hh
                            qps = qkp.tile([128, 512], F32, tag="qk")
                            for dc in range(16):
                                nc.tensor.matmul(
                                    qps[:], wq_sb[:, dc, hh*128:(hh+1)*128],
                                    xin[:, dc, :], start=(dc == 0), stop=(dc == 15))
                            q_sb = spool.tile([128, 512], F32R, tag="qko")
                            nc.scalar.activation(q_sb[:], qps[:], AF.Identity,
                                                 bias=bq_sb[:, h:h+1], scale=1.0)
                            nc.sync.dma_start(out=qd[h][:, sc*512:(sc+1)*512], in_=q_sb[:])

                            kps = qkp.tile([128, 512], F32, tag="qk")
                            for dc in range(16):
                                nc.tensor.matmul(
                                    kps[:], wk_sb[:, dc, hh*128:(hh+1)*128],
                                    xin[:, dc, :], start=(dc == 0), stop=(dc == 15))
                            k_sb = spool.tile([128, 512], F32R, tag="qko")
                            nc.scalar.activation(k_sb[:], kps[:], AF.Identity,
                                                 bias=bk_sb[:, h:h+1], scale=1.0)
                            nc.sync.dma_start(out=kd[h][:, sc*512:(sc+1)*512], in_=k_sb[:])
                        # V for this pass: [k, dv] chunks (4 heads' dv)
                        for kc in range(4):
                            vp = vps.tile([128, 512], F32, tag="v")
                            for dc in range(16):
                                nc.tensor.matmul(
                                    vp[:], xin[:, dc, kc*128:(kc+1)*128],
                                    wv_sb[:, dc, :], start=(dc == 0), stop=(dc == 15))
                            v_sb = spool.tile([128, 512], F32R, tag="vo")
                            nc.vector.tensor_add(v_sb[:], vp[:], bv_sb[:, p_, :])
                            for hh in range(4):
                                h = p_ * 4 + hh
                                nc.sync.dma_start(
                                    out=vd[h][:, sc*4+kc, :],
                                    in_=v_sb[:, hh*128:(hh+1)*128])

            # ------------- Stage 2: attention (head-outer) -------------
            # K/V/Q loaded once per head; ctx spilled to DRAM; out-projection
            # runs as a final dense phase with all four Wo e-chunks resident.
            ctxd = dpool.tile([128, HPC, NSC, 512], F32R, name="ctxd")
            units = [(h, qc) for h in range(HPC) for qc in range(NSC)]
            with tc.tile_pool(name="s2kv", bufs=3) as kvpool, \
                 tc.tile_pool(name="s2es", bufs=2) as espool, \
                 tc.tile_pool(name="s2sm", bufs=2) as smpool, \
                 tc.tile_pool(name="s3wo", bufs=2) as wopool, \
                 tc.tile_pool(name="s3cx", bufs=2) as cx3pool, \
                 tc.tile_pool(name="s3o", bufs=2) as opool, \
                 tc.tile_pool(name="psp", bufs=3, space="PSUM") as psp, \
                 tc.tile_pool(name="pcd", bufs=2, space="PSUM") as pcd, \
                 tc.tile_pool(name="pdt", bufs=1, space="PSUM") as pdt, \
                 tc.tile_pool(name="pop", bufs=2, space="PSUM") as pop, \
                 tc.tile_pool(name="rcpdram", bufs=2, space="DRAM") as rdpool:
                state = {}
                hstate = {}
                pending = []
                for it in range(len(units) + 2):
                    if it < len(units):
                        h, qc = units[it]
                        nk = 4 * qc + 4
                        if qc == 0:
                            k2 = kvpool.tile([128, seq], F32R, tag="k2")
                            nc.sync.dma_start(out=k2[:], in_=kd[h][:])
                            v2 = kvpool.tile([128, NKC, 128], F32R, tag="v2")
                            nc.sync.dma_start(out=v2[:], in_=vd[h][:])
                            acc4 = smpool.tile([128, NSC, 512], F32, tag="acc4")
                            ctxu4 = smpool.tile([128, NSC, 512], F32, tag="ctxu4")
                            hstate[h] = (k2, v2, acc4, ctxu4)
                        k2, v2, acc4, ctxu4 = hstate[h]
                        q3 = kvpool.tile([128, 512], F32R, tag="q3")
                        nc.sync.dma_start(out=q3[:], in_=qd[h][:, qc*512:(qc+1)*512])
                        es = espool.tile([128, NKC, 512], F32R, tag="es")
                        for kc in range(nk):
                            j = kc - 4 * qc
                            lo = 128 * j if j > 0 else 0
                            sp = psp.tile([128, 512], F32, tag="sp")
                            nc.tensor.matmul(
                                sp[:], k2[:, kc*128:(kc+1)*128], q3[:],
                                start=True, stop=True)
                            nc.scalar.activation(es[:, kc, lo:], sp[:, lo:],
                                                 AF.Exp, bias=0.0, scale=SCALE)
                        state[(h, qc)] = es
                    if 0 <= it - 1 < len(units):
                        h, qc = units[it - 1]
                        nk = 4 * qc + 4
                        k2, v2, acc4, ctxu4 = hstate[h]
                        es = state.pop((h, qc))
                        ctxp = pcd.tile([128, 512], F32, tag="cd")
                        for kc in range(nk):
                            j = kc - 4 * qc
                            if j >= 0:   # diagonal tile: causal mask
                                # fills the un-exp'd prefix (cols < 128j) too
                                nc.gpsimd.affine_select(
                                    out=es[:, kc, :], in_=es[:, kc, :],
                                    compare_op=mybir.AluOpType.is_ge,
                                    fill=0.0, base=-128 * j,
                                    pattern=[[1, 512]], channel_multiplier=-1)
                            if kc == 0:
                                nc.vector.tensor_copy(
                                    acc4[:, qc, :], es[:, kc, :].bitcast(F32))
                            else:
                                nc.vector.tensor_add(
                                    acc4[:, qc, :], acc4[:, qc, :],
                                    es[:, kc, :].bitcast(F32))
                            nc.tensor.matmul(
                                ctxp[:], v2[:, kc, :], es[:, kc, :],
                                start=(kc == 0), stop=(kc == nk - 1))
                        nc.scalar.activation(ctxu4[:, qc, :], ctxp[:], AF.Copy)
                        if qc == NSC - 1:
                            pending.append((h, it))
                    if pending and pending[0][1] < it:
                        # head h' finished >=1 full step ago: denominators
                        h2, _ = pending.pop(0)
                        _, _, acc4, ctxu4 = hstate.pop(h2)
                        dent = pdt.tile([128, NSC * 8], F32, tag="dent")
                        for qc2 in range(NSC):
                            for qs in range(4):
                                c = 2 * (qc2 * 4 + qs)
                                nc.tensor.matmul(
                                    dent[:, c:c+2],
                                    acc4[:, qc2, qs*128:(qs+1)*128],
                                    ones_f[:, 0:2], start=True, stop=True)
                        rcpt = smpool.tile([128, NSC * 8], F32, tag="rcpt")
                        nc.vector.reciprocal(rcpt[:], dent[:])
                        rcpd = rdpool.tile([NSC, 4, 128], F32, tag="rcpd")
                        nc.sync.dma_start(
                            out=rcpd.rearrange("qc qs p -> p (qc qs)"),
                            in_=rcpt.rearrange("p (a b) -> p a b", b=2)[:, :, 0])
                        for qc2 in range(NSC):
                            rcpb = smpool.tile([128, 512], F32, tag="rcpb")
                            nc.sync.dma_start(
                                out=rcpb[:],
                                in_=rcpd[qc2].rearrange("qs p -> (qs p)")
                                    .partition_broadcast(128))
                            ctxn = smpool.tile([128, 512], F32R, tag="ctxn")
                            nc.vector.tensor_mul(ctxn[:], ctxu4[:, qc2, :], rcpb[:])
                            nc.sync.dma_start(out=ctxd[:, h2, qc2, :], in_=ctxn[:])

                # ---- out-projection (same pool scope: overlaps tail) ----
                for ecp in range(2):
                    wos = []
                    for ei in range(2):
                        wo_sb = wopool.tile([128, HPC, 512], F32R, tag="wo")
                        ec = ecp * 2 + ei
                        nc.sync.dma_start(out=wo_sb[:],
                                          in_=wot_r[:, :, ec*512:(ec+1)*512])
                        wos.append(wo_sb)
                    for qc in range(NSC):
                        for ss in range(4):
                            ctxs = cx3pool.tile([128, HPC, 128], F32R, tag="ctxs")
                            nc.sync.dma_start(
                                out=ctxs[:], in_=ctxd[:, :, qc, ss*128:(ss+1)*128])
                            ops = [pop.tile([128, 512], F32, tag="op",
                                            name=f"op{ei}") for ei in range(2)]
                            for h in range(HPC):
                                for ei in range(2):
                                    nc.tensor.matmul(
                                        ops[ei][:], ctxs[:, h, :], wos[ei][:, h, :],
                                        start=(h == 0), stop=(h == HPC - 1))
                            for ei in range(2):
                                ec = ecp * 2 + ei
                                o_sb = opool.tile([128, 512], F32, tag="o")
                                nc.scalar.activation(o_sb[:], ops[ei][:], AF.Copy)
                                nc.sync.dma_start(
                                    out=out[qc*512+ss*128:qc*512+(ss+1)*128,
                                            ec*512:(ec+1)*512],
                                    in_=o_sb[:])
    split_excess_waits(nc)
    return nc


_NC_CACHE = {}


def _get_nc(seq):
    if seq not in _NC_CACHE:
        _NC_CACHE[seq] = build_nc(seq)
    return _NC_CACHE[seq]


def make_in_maps(x, Wq, bq, Wk, bk, Wv, bv, Wo, bo, seq=S, nb=B):
    f32 = np.float32
    in_maps = []
    for c in range(NCORES):
        b = c // 2
        half = c % 2
        sl = slice(half * MLOC, (half + 1) * MLOC)
        in_maps.append({
            "xt": np.ascontiguousarray(x[b].T, dtype=f32),
            "wqt": np.ascontiguousarray(Wq[sl, :].T, dtype=f32),
            "wkt": np.ascontiguousarray(Wk[sl, :].T, dtype=f32),
            "wvt": np.ascontiguousarray(Wv[sl, :].T, dtype=f32),
            "wot": np.ascontiguousarray(Wo[:, sl].T, dtype=f32),
            "bqt": np.ascontiguousarray(bq[sl].reshape(HPC, DK).T, dtype=f32),
            "bkt": np.ascontiguousarray(bk[sl].reshape(HPC, DK).T, dtype=f32),
            "bvv": np.ascontiguousarray(bv[sl], dtype=f32),
            "ones": np.ones((DK, DK), dtype=f32),
        })
    return in_maps


def run(inputs, trace=False, trace_kwargs=None):
    x = np.asarray(inputs["x"], dtype=np.float32)
    nb, seq, d = x.shape
    nc = _get_nc(seq)
    in_maps = make_in_maps(
        x, np.asarray(inputs["Wq"]), np.asarray(inputs["bq"]),
        np.asarray(inputs["Wk"]), np.asarray(inputs["bk"]),
        np.asarray(inputs["Wv"]), np.asarray(inputs["bv"]),
        np.asarray(inputs["Wo"]), np.asarray(inputs["bo"]), seq=seq, nb=nb)
    res = run_bass_kernel_spmd(nc, in_maps, list(range(NCORES)), trace=trace,
                               **(trace_kwargs or {}))
    bo = np.asarray(inputs["bo"], dtype=np.float32)
    out = np.empty((nb, seq, d), dtype=np.float32)
    for b in range(nb):
        out[b] = res.results[2*b]["out"] + res.results[2*b+1]["out"] + bo
    return out, res


def kernel(**inputs):
    out, _ = run(inputs, trace=False)
    return out



# revision 5
# speedup vs baseline: 1.3035x; 1.3035x over previous
"""Causal self-attention (B=4, S=2048, D=2048, H=16) on 8 Trainium2 cores.

Sharding: core c -> (batch b = c//2, head-half = c%2, i.e. 8 of 16 heads).
Megatron-style: Wq/Wk/Wv column-parallel (8 heads' rows), Wo row-parallel
(matching 1024 columns).  Each core emits a partial (S, D) output for its
batch; host sums the two half partials per batch and adds bo.

v2 design (vs baseline): all matmul operands in bf16 (same PE rate as
fp32r, half the DMA/SBUF), K^T/Q^T/V SBUF-resident end to end (no DRAM
spill round-trips), stage 2 runs qc-outer/head-inner with the
out-projection fused per q-chunk (dense PE work hides ACT/DVE softmax
tail), softmax denominator via ones-matmul partition-reduce+broadcast
plus DVE reciprocal (no DMA broadcast round-trip), and causal-trimmed
diagonal score/context matmuls.  fp32 PSUM accumulation everywhere.

Device pipeline per core (S=2048, DK=128, 8 local heads):
  Stage 1 (two 4-head passes): QKV projections.
    Q^T,K^T per head in [dk, s]; V in [s, dv] chunks -> all in SBUF bf16.
  Stage 2 per q-chunk (512), heads inner:
    S^T tile [k,q] = K^T_chunk.T @ Q^T   (bf16, diag tiles suffix-only)
    es = Exp(S^T / sqrt(dk)) -> bf16     (no max-subtraction; scores~N(0,1))
    causal mask: gpsimd affine_select on diag tiles
    acc[k,q] += es (DVE, fp32); den bcast = ones.T @ acc (PE);
    rcp = 1/den (DVE); ctx^T [dv,q] = sum_k V_chunk.T @ es (PE, fp32 PSUM)
    ctx_sb = ctx^T * rcp -> bf16 (DVE)
    out-proj for the q-chunk: out[q,e] partial = sum_{h} ctx_sb_h.T @ WoT_h
    accumulated over all 8 heads in PSUM, DMA'd straight to DRAM.
"""

import math

import numpy as np

import concourse.bass as bass
import concourse.mybir as mybir
from concourse.bass_utils import run_bass_kernel_spmd
from concourse.tile import TileContext

B, S, D, H = 4, 2048, 2048, 16
DK = 128
NCORES = 8
HPC = H // 2          # 8 heads per core
MLOC = HPC * DK       # 1024 local head dims

F32 = mybir.dt.float32
F32R = mybir.dt.float32r
BF16 = mybir.dt.bfloat16
AF = mybir.ActivationFunctionType


def split_excess_waits(nc, max_waits=1):
    """walrus in this container accepts at most one sem-wait per instruction;
    move excess waits onto wait-only EventSemaphore insts inserted before."""
    ctr = 0
    for f in nc.m.functions:
        for bb in f.blocks:
            new = []
            changed = False
            for inst in bb.instructions:
                si = inst.sync_info
                if si is not None and si.on_wait and len(si.on_wait) > max_waits:
                    changed = True
                    waits = list(si.on_wait)
                    for w in waits[:-max_waits]:
                        ctr += 1
                        ev = mybir.InstEventSemaphore(
                            name=f"waitsplit-{ctr}", ins=[], outs=[],
                            sync_info=mybir.SyncInfo(on_wait=[w], on_update=[]))
                        ev.engine = inst.engine
                        new.append(ev)
                    si.on_wait = waits[-max_waits:]
                new.append(inst)
            if changed:
                bb.instructions = new
    return ctr


def build_nc(seq=S):
    """One core's program: full attention for 1 batch x 8 heads."""
    assert seq % 512 == 0
    NSC = seq // 512          # 512-wide q chunks
    NKC = seq // 128          # 128-wide k chunks
    SCALE = 1.0 / math.sqrt(DK)

    nc = bass.Bass()
    xt = nc.declare_dram_parameter("xt", [D, seq], BF16, isOutput=False)
    wqt = nc.declare_dram_parameter("wqt", [D, MLOC], BF16, isOutput=False)
    wkt = nc.declare_dram_parameter("wkt", [D, MLOC], BF16, isOutput=False)
    wvt = nc.declare_dram_parameter("wvt", [D, MLOC], BF16, isOutput=False)
    wot = nc.declare_dram_parameter("wot", [MLOC, D], BF16, isOutput=False)
    bqt = nc.declare_dram_parameter("bqt", [DK, HPC], F32, isOutput=False)
    bkt = nc.declare_dram_parameter("bkt", [DK, HPC], F32, isOutput=False)
    bvv = nc.declare_dram_parameter("bvv", [MLOC], F32, isOutput=False)
    ones = nc.declare_dram_parameter("ones", [DK, DK], F32R, isOutput=False)
    out = nc.declare_dram_parameter("out", [seq, D], F32, isOutput=True)

    xt_r = xt.rearrange("(dc p) s -> p dc s", p=128)      # [128, 16, seq]
    wqt_r = wqt.rearrange("(dc p) m -> p dc m", p=128)    # [128, 16, 1024]
    wkt_r = wkt.rearrange("(dc p) m -> p dc m", p=128)
    wvt_r = wvt.rearrange("(dc p) m -> p dc m", p=128)
    wot_r = wot.rearrange("(hc p) e -> p hc e", p=128)    # [128, 8, 2048]

    with TileContext(nc) as tc:
        with tc.tile_pool(name="big", bufs=1) as bpool, \
             tc.tile_pool(name="const", bufs=1) as cpool:
            # SBUF-resident per-head tensors (bf16)
            kT = bpool.tile([128, HPC, seq], BF16, name="kT")    # [dk, h, s]
            qT = bpool.tile([128, HPC, seq], BF16, name="qT")    # [dk, h, s]
            vA = bpool.tile([128, NKC, HPC, 128], BF16, name="vA")  # [s, kc, h, dv]

            ones_sb = cpool.tile([DK, DK], F32R)
            nc.sync.dma_start(out=ones_sb[:], in_=ones[:])
            bq_sb = cpool.tile([DK, HPC], F32)
            nc.sync.dma_start(out=bq_sb[:], in_=bqt[:])
            bk_sb = cpool.tile([DK, HPC], F32)
            nc.sync.dma_start(out=bk_sb[:], in_=bkt[:])
            bv_sb = cpool.tile([128, 2, 512], F32)
            for p_ in range(2):
                nc.sync.dma_start(
                    out=bv_sb[:, p_, :],
                    in_=bvv[p_*512:(p_+1)*512].partition_broadcast(128))

            # ---------------- Stage 1: QKV projections ----------------
            with tc.tile_pool(name="s1w", bufs=1) as wpool, \
                 tc.tile_pool(name="s1x", bufs=2) as xpool, \
                 tc.tile_pool(name="s1qk", bufs=3, space="PSUM") as qkp, \
                 tc.tile_pool(name="s1v", bufs=2, space="PSUM") as vps:
                for p_ in range(2):           # head-half pass: heads 4p..4p+3
                    wq_sb = wpool.tile([128, 16, 512], BF16, tag="wq")
                    wk_sb = wpool.tile([128, 16, 512], BF16, tag="wk")
                    wv_sb = wpool.tile([128, 16, 512], BF16, tag="wv")
                    # chunked loads so first matmuls start after ~1/4 of data
                    for c4 in range(4):
                        nc.sync.dma_start(
                            out=wq_sb[:, c4*4:(c4+1)*4, :],
                            in_=wqt_r[:, c4*4:(c4+1)*4, p_*512:(p_+1)*512])
                    for c4 in range(4):
                        nc.sync.dma_start(
                            out=wk_sb[:, c4*4:(c4+1)*4, :],
                            in_=wkt_r[:, c4*4:(c4+1)*4, p_*512:(p_+1)*512])
                    for c4 in range(4):
                        nc.sync.dma_start(
                            out=wv_sb[:, c4*4:(c4+1)*4, :],
                            in_=wvt_r[:, c4*4:(c4+1)*4, p_*512:(p_+1)*512])
                    for sc in range(NSC):
                        xin = xpool.tile([128, 16, 512], BF16, tag="xin")
                        for c4 in range(4):
                            nc.sync.dma_start(
                                out=xin[:, c4*4:(c4+1)*4, :],
                                in_=xt_r[:, c4*4:(c4+1)*4, sc*512:(sc+1)*512])
                        for hh in range(4):
                            h = p_ * 4 + hh
                            qps = qkp.tile([128, 512], F32, tag="qk")
                            for dc in range(16):
                                nc.tensor.matmul(
                                    qps[:], wq_sb[:, dc, hh*128:(hh+1)*128],
                                    xin[:, dc, :], start=(dc == 0), stop=(dc == 15))
                            nc.scalar.activation(qT[:, h, sc*512:(sc+1)*512],
                                                 qps[:], AF.Identity,
                                                 bias=bq_sb[:, h:h+1], scale=1.0)

                            kps = qkp.tile([128, 512], F32, tag="qk")
                            for dc in range(16):
                                nc.tensor.matmul(
                                    kps[:], wk_sb[:, dc, hh*128:(hh+1)*128],
                                    xin[:, dc, :], start=(dc == 0), stop=(dc == 15))
                            nc.scalar.activation(kT[:, h, sc*512:(sc+1)*512],
                                                 kps[:], AF.Identity,
                                                 bias=bk_sb[:, h:h+1], scale=1.0)
                        # V for this pass: [k, dv] chunks (4 heads' dv)
                        for kc in range(4):
                            vp = vps.tile([128, 512], F32, tag="v")
                            for dc in range(16):
                                nc.tensor.matmul(
                                    vp[:], xin[:, dc, kc*128:(kc+1)*128],
                                    wv_sb[:, dc, :], start=(dc == 0), stop=(dc == 15))
                            nc.vector.tensor_add(
                                vA[:, sc*4+kc, p_*4:(p_+1)*4, :],
                                vp[:].rearrange("p (h v) -> p h v", v=128),
                                bv_sb[:, p_, :].rearrange("p (h v) -> p h v", v=128))

            # ------- Stage 2+3: attention + fused out-projection -------
            with tc.tile_pool(name="s2wo", bufs=1) as wopool, \
                 tc.tile_pool(name="s2es", bufs=2) as espool, \
                 tc.tile_pool(name="s2acc", bufs=2) as accpool, \
                 tc.tile_pool(name="s2rcp", bufs=2) as rcpool, \
                 tc.tile_pool(name="s2cx", bufs=2) as cxpool, \
                 tc.tile_pool(name="s3o", bufs=3) as opool, \
                 tc.tile_pool(name="psp", bufs=2, space="PSUM") as psp, \
                 tc.tile_pool(name="pcd", bufs=2, space="PSUM") as pcd, \
                 tc.tile_pool(name="pdt", bufs=1, space="PSUM") as pdt, \
                 tc.tile_pool(name="pop", bufs=3, space="PSUM") as pop:
                wo_sb = wopool.tile([128, HPC, D], BF16, name="wo_sb")
                for c4 in range(4):
                    nc.sync.dma_start(
                        out=wo_sb[:, c4*2:(c4+1)*2, :],
                        in_=wot_r[:, c4*2:(c4+1)*2, :])
                for qc in range(NSC):
                    nk = 4 * qc + 4
                    ctx_sb = cxpool.tile([128, HPC, 512], BF16, tag="ctxq")
                    for h in range(HPC):
                        es = espool.tile([128, NKC, 512], BF16, tag="es")
                        acc = accpool.tile([128, 512], F32R, tag="acc")
                        ctxp = pcd.tile([128, 512], F32, tag="cd")
                        for kc in range(nk):
                            j = kc - 4 * qc
                            lo = 128 * j if j > 0 else 0
                            sp = psp.tile([128, 512], F32, tag="sp")
                            nc.tensor.matmul(
                                sp[:, lo:], kT[:, h, kc*128:(kc+1)*128],
                                qT[:, h, qc*512+lo:(qc+1)*512],
                                start=True, stop=True)
                            nc.scalar.activation(es[:, kc, lo:], sp[:, lo:],
                                                 AF.Exp, bias=0.0, scale=SCALE)
                            if j >= 0:   # diagonal tile: causal mask
                                nc.gpsimd.affine_select(
                                    out=es[:, kc, lo:], in_=es[:, kc, lo:],
                                    compare_op=mybir.AluOpType.is_ge,
                                    fill=0.0, base=0,
                                    pattern=[[1, 512 - lo]],
                                    channel_multiplier=-1)
                            if kc == 0:
                                nc.vector.tensor_copy(acc[:], es[:, 0, :])
                            else:
                                nc.vector.tensor_add(
                                    acc[:, lo:], acc[:, lo:], es[:, kc, lo:])
                            nc.tensor.matmul(
                                ctxp[:, lo:], vA[:, kc, h, :], es[:, kc, lo:],
                                start=(kc == 0), stop=(kc == nk - 1))
                        dent = pdt.tile([128, 512], F32, tag="dent")
                        nc.tensor.matmul(dent[:], ones_sb[:], acc[:],
                                         start=True, stop=True)
                        rcpb = rcpool.tile([128, 512], F32, tag="rcpb")
                        nc.vector.reciprocal(rcpb[:], dent[:])
                        nc.vector.tensor_mul(ctx_sb[:, h, :], ctxp[:], rcpb[:])
                    # ---- out-projection for this q-chunk ----
                    for ss in range(4):
                        for ei in range(4):
                            ops = pop.tile([128, 512], F32, tag="op")
                            for h in range(HPC):
                                nc.tensor.matmul(
                                    ops[:], ctx_sb[:, h, ss*128:(ss+1)*128],
                                    wo_sb[:, h, ei*512:(ei+1)*512],
                                    start=(h == 0), stop=(h == HPC - 1))
                            o_sb = opool.tile([128, 512], F32, tag="o")
                            nc.scalar.activation(o_sb[:], ops[:], AF.Copy)
                            nc.sync.dma_start(
                                out=out[qc*512+ss*128:qc*512+(ss+1)*128,
                                        ei*512:(ei+1)*512],
                                in_=o_sb[:])
    split_excess_waits(nc)
    return nc


_NC_CACHE = {}


def _get_nc(seq):
    if seq not in _NC_CACHE:
        _NC_CACHE[seq] = build_nc(seq)
    return _NC_CACHE[seq]


def make_in_maps(x, Wq, bq, Wk, bk, Wv, bv, Wo, bo, seq=S, nb=B):
    import ml_dtypes
    bf16 = ml_dtypes.bfloat16
    f32 = np.float32
    in_maps = []
    for c in range(NCORES):
        b = c // 2
        half = c % 2
        sl = slice(half * MLOC, (half + 1) * MLOC)
        in_maps.append({
            "xt": np.ascontiguousarray(x[b].T.astype(bf16)),
            "wqt": np.ascontiguousarray(Wq[sl, :].T.astype(bf16)),
            "wkt": np.ascontiguousarray(Wk[sl, :].T.astype(bf16)),
            "wvt": np.ascontiguousarray(Wv[sl, :].T.astype(bf16)),
            "wot": np.ascontiguousarray(Wo[:, sl].T.astype(bf16)),
            "bqt": np.ascontiguousarray(bq[sl].reshape(HPC, DK).T, dtype=f32),
            "bkt": np.ascontiguousarray(bk[sl].reshape(HPC, DK).T, dtype=f32),
            "bvv": np.ascontiguousarray(bv[sl], dtype=f32),
            "ones": np.ones((DK, DK), dtype=f32),
        })
    return in_maps


def run(inputs, trace=False, trace_kwargs=None):
    x = np.asarray(inputs["x"], dtype=np.float32)
    nb, seq, d = x.shape
    nc = _get_nc(seq)
    in_maps = make_in_maps(
        x, np.asarray(inputs["Wq"]), np.asarray(inputs["bq"]),
        np.asarray(inputs["Wk"]), np.asarray(inputs["bk"]),
        np.asarray(inputs["Wv"]), np.asarray(inputs["bv"]),
        np.asarray(inputs["Wo"]), np.asarray(inputs["bo"]), seq=seq, nb=nb)
    res = run_bass_kernel_spmd(nc, in_maps, list(range(NCORES)), trace=trace,
                               **(trace_kwargs or {}))
    bo = np.asarray(inputs["bo"], dtype=np.float32)
    out = np.empty((nb, seq, d), dtype=np.float32)
    for b in range(nb):
        out[b] = res.results[2*b]["out"] + res.results[2*b+1]["out"] + bo
    return out, res


def kernel(**inputs):
    out, _ = run(inputs, trace=False)
    return out
